# revision 33
# baseline (speedup 1.0000x reference)
"""LoRA linear kernel for Trainium2 (8 NeuronCores, SPMD data-parallel).

Computes out = x @ (A @ B) for
    x: [4, 2048, 4096] f32, A: [4096, 16] f32, B: [16, 4096] f32
by reassociating to (x @ A) @ B  (4.3 GFLOP instead of 274 GFLOP).

Sharding: x is split row-wise (batch*seq = 8192 rows -> 1024 rows/core),
A and B are replicated; no collectives.  Per core:

  stage 1:  tT[16, n]   = sum_c aP_c[128,16].T @ xP_c[128, n]   (PSUM accum)
  stage 2:  out[128, d] = tT[:, rb].T @ B[16, d]                (per 512-col)

The host pre-packs x into xP[p, rc, c, n] = x[row, c*128+p] so every
per-chunk DMA is one contiguous per-partition block (line-rate HBM), and
A into aP[p, c, j] = A[c*128+p, j].  Matmuls run as float32r (full-rate
fp32 mode) by default; bf16 variants trade a little precision for half
the HBM traffic.  Input loads issue on the Sync HWDGE ring, output
stores on the Scalar ring so they don't head-of-line block each other.
"""

import numpy as np
import ml_dtypes

import concourse.bass as bass  # noqa: F401  (kept for parity with docs)
import concourse.bacc as bacc
import concourse.mybir as mybir
from concourse.tile import TileContext
from concourse.bass_utils import run_bass_kernel_spmd

N_CORES = 8
BATCH, SEQ, D_IN, D_OUT, R = 4, 2048, 4096, 4096, 16
ROWS = BATCH * SEQ              # 8192
RPC = ROWS // N_CORES           # 1024 rows per core
KC = D_IN // 128                # 32 contraction chunks of 128
DC = 512                        # d_out columns per stage-2 matmul (PSUM bank)
NDC = D_OUT // DC               # 8

F32 = mybir.dt.float32
FR = mybir.dt.float32r
BF16 = mybir.dt.bfloat16

_NPDT = {str(F32): np.float32, str(FR): np.float32,
         str(BF16): ml_dtypes.bfloat16}

_cache = {}


def _build_v2(mm_dtype, out_dtype, rchunk, copy_split, obufs, fast_start=0,
              store_split=1, warmup=0, wide_po=False, tail_sync=False,
              tt_act=False, store_rings=1, const_sync=False, dual_load=False):
    nch = RPC // rchunk
    nc = bacc.Bacc("TRN2", target_bir_lowering=False)
    xP = nc.dram_tensor("xP", [128, nch, KC, rchunk], mm_dtype,
                        kind="ExternalInput")
    aP = nc.dram_tensor("aP", [128, KC, R], mm_dtype, kind="ExternalInput")
    Bw = nc.dram_tensor("Bw", [R, D_OUT], mm_dtype, kind="ExternalInput")
    out = nc.dram_tensor("out", [RPC, D_OUT], out_dtype, kind="ExternalOutput")

    po_bufs = 3 if wide_po else 6

    with TileContext(nc) as tc:
        with (
            tc.tile_pool(name="consts", bufs=1) as cpool,
            tc.tile_pool(name="xin", bufs=2 * max(fast_start, 1)) as xpool,
            tc.tile_pool(name="tbuf", bufs=2) as tpool,
            tc.tile_pool(name="obuf", bufs=obufs) as opool,
            tc.tile_pool(name="pt", bufs=2, space="PSUM") as ptpool,
            tc.tile_pool(name="po", bufs=po_bufs, space="PSUM") as popool,
        ):
            nsp0 = max(fast_start, 1)
            kcs0 = KC // nsp0
            xt00 = None
            if const_sync:
                # Interleave on the sync ring: A (tiny) -> first x half-chunk
                # -> B.  A gates the first matmul, B only gates stage-2, and
                # the gpsimd SWDGE path turned out to have ~10us of launch
                # latency that delayed everything downstream.
                a_tile = cpool.tile([128, KC, R], mm_dtype)
                nc.sync.dma_start(out=a_tile[:], in_=aP[:, :, :])
                xt00 = xpool.tile([128, kcs0, rchunk], mm_dtype,
                                  name="xt", tag="xt")
                nc.sync.dma_start(out=xt00[:], in_=xP[:, 0, 0:kcs0, :])
                b_tile = cpool.tile([R, D_OUT], mm_dtype)
                nc.sync.dma_start(out=b_tile[:], in_=Bw[:, :])
            else:
                # constants go on the gpsimd SWDGE queue so the sync HWDGE
                # ring is free for the x stream from instruction 0.
                cdma = nc.gpsimd if fast_start else nc.sync
                a_tile = cpool.tile([128, KC, R], mm_dtype)
                cdma.dma_start(out=a_tile[:], in_=aP[:, :, :])
                b_tile = cpool.tile([R, D_OUT], mm_dtype)
                cdma.dma_start(out=b_tile[:], in_=Bw[:, :])

            if warmup:
                # Keep the PE busy while the first x chunk streams in, so the
                # HAM clock gate is already released (2.4 GHz) when real
                # matmuls start.  Source data is a zeroed SBUF tile; results
                # go to a scratch PSUM slot (shared with the po pool) that
                # nobody reads.
                wsrc = cpool.tile([128, DC], mm_dtype)
                nc.vector.memset(wsrc[:], 0.0)
                wdst = popool.tile([128, DC], F32, name="wdst", tag="po")
                for _ in range(warmup):
                    nc.tensor.matmul(wdst[:], wsrc[:, :128], wsrc[:],
                                     start=True, stop=True)

            nsp = max(fast_start, 1)
            kcs = KC // nsp

            for rc in range(nch):
                # stage 1: tT [16, rchunk] = (x_chunk @ A).T via PSUM accum.
                # With fast_start the chunk load is split so matmuls start
                # after the first fraction lands.
                pt = ptpool.tile([R, rchunk], F32)
                for h in range(nsp):
                    if rc == 0 and h == 0 and xt00 is not None:
                        xt = xt00
                    else:
                        xt = xpool.tile([128, kcs, rchunk], mm_dtype,
                                        name="xt", tag="xt")
                        # dual_load ping-pongs halves between the sync HWDGE
                        # ring and the gpsimd SWDGE queue so two input DMAs
                        # are in flight at once.
                        if dual_load and (rc * nsp + h) % 2 == 1:
                            ldma = nc.gpsimd
                        else:
                            ldma = nc.sync
                        ldma.dma_start(
                            out=xt[:], in_=xP[:, rc, h * kcs:(h + 1) * kcs, :])
                    for c in range(kcs):
                        nc.tensor.matmul(
                            pt[:],
                            a_tile[:, h * kcs + c, :],
                            xt[:, c, :],
                            start=(h == 0 and c == 0),
                            stop=(h == nsp - 1 and c == kcs - 1),
                        )
                tT = tpool.tile([R, rchunk], mm_dtype)
                if tt_act:
                    nc.scalar.copy(out=tT[:], in_=pt[:])
                else:
                    nc.vector.tensor_copy(tT[:], pt[:])

                # stage 2: out rows = tT.T @ B, one 128-row block at a time
                for rb in range(rchunk // 128):
                    if store_rings == 2:
                        sdma = nc.scalar if (rc * 2 + rb) % 2 == 0 else nc.sync
                    elif tail_sync and rc == nch - 1:
                        sdma = nc.sync
                    else:
                        sdma = nc.scalar
                    row0 = rc * rchunk + rb * 128
                    osb = opool.tile([128, D_OUT], out_dtype)
                    if wide_po:
                        for dcp in range(NDC // 2):
                            po = popool.tile([128, 2 * DC], F32)
                            for half in range(2):
                                d0 = (2 * dcp + half) * DC
                                nc.tensor.matmul(
                                    po[:, half * DC:(half + 1) * DC],
                                    tT[:, rb * 128:(rb + 1) * 128],
                                    b_tile[:, d0:d0 + DC],
                                    start=True,
                                    stop=True,
                                )
                            d0 = 2 * dcp * DC
                            if copy_split and dcp % copy_split == copy_split - 1:
                                nc.scalar.copy(
                                    out=osb[:, d0:d0 + 2 * DC], in_=po[:])
                            else:
                                nc.vector.tensor_copy(
                                    osb[:, d0:d0 + 2 * DC], po[:])
                            if store_split > 1 and dcp % 2 == 1:
                                s = (dcp // 2) * (D_OUT // 2)
                                seg = D_OUT // 2
                                sdma.dma_start(
                                    out=out[row0:row0 + 128, s:s + seg],
                                    in_=osb[:, s:s + seg])
                    else:
                        for dc in range(NDC):
                            po = popool.tile([128, DC], F32)
                            nc.tensor.matmul(
                                po[:],
                                tT[:, rb * 128:(rb + 1) * 128],
                                b_tile[:, dc * DC:(dc + 1) * DC],
                                start=True,
                                stop=True,
                            )
                            if copy_split and dc % copy_split == copy_split - 1:
                                nc.scalar.copy(
                                    out=osb[:, dc * DC:(dc + 1) * DC],
                                    in_=po[:])
                            else:
                                nc.vector.tensor_copy(
                                    osb[:, dc * DC:(dc + 1) * DC], po[:])
                            if (store_split > 1
                                    and (dc + 1) % (NDC // store_split) == 0):
                                seg = D_OUT // store_split
                                s = ((dc + 1) // (NDC // store_split) - 1) * seg
                                sdma.dma_start(
                                    out=out[row0:row0 + 128, s:s + seg],
                                    in_=osb[:, s:s + seg])
                    if store_split <= 1:
                        sdma.dma_start(out=out[row0:row0 + 128, :],
                                       in_=osb[:])
    nc.compile()
    return nc


def _build_v3(mm_dtype, out_dtype, nway, copy_split, nsplit):
    """Strip-packed: nway 128-row blocks processed concurrently on disjoint
    32-partition strips of the PE array (stage-1 in column strips, stage-2
    in row strips), so the 16-wide LoRA contraction doesn't leave the PE
    array 87% idle."""
    rchunk = 128 * nway
    nch = RPC // rchunk
    nc = bacc.Bacc("TRN2", target_bir_lowering=False)
    xP = nc.dram_tensor("xP", [128, nch, KC, rchunk], mm_dtype,
                        kind="ExternalInput")
    aP = nc.dram_tensor("aP", [128, KC, R], mm_dtype, kind="ExternalInput")
    Bw = nc.dram_tensor("Bw", [R, D_OUT], mm_dtype, kind="ExternalInput")
    out = nc.dram_tensor("out", [RPC, D_OUT], out_dtype, kind="ExternalOutput")

    kcs = KC // nsplit

    with TileContext(nc) as tc:
        with (
            tc.tile_pool(name="consts", bufs=1) as cpool,
            tc.tile_pool(name="xin", bufs=2 * nsplit) as xpool,
            tc.tile_pool(name="tbuf", bufs=2) as tpool,
            tc.tile_pool(name="obuf", bufs=min(2 * nway, 6)) as opool,
            tc.tile_pool(name="pt", bufs=2, space="PSUM") as ptpool,
            tc.tile_pool(name="po", bufs=6, space="PSUM") as popool,
        ):
            a_tile = cpool.tile([128, KC, R], mm_dtype)
            nc.sync.dma_start(out=a_tile[:], in_=aP[:, :, :])
            # B replicated into partition strips 32g..32g+16
            b4 = cpool.tile([128, D_OUT], mm_dtype)
            for g in range(nway):
                nc.sync.dma_start(out=b4[32 * g:32 * g + R, :], in_=Bw[:, :])

            for rc in range(nch):
                # stage 1: nway concurrent col-strip matmuls; strip g
                # accumulates tT of row-block g into psum rows 32g..32g+16.
                pt = ptpool.tile([128, 128], F32)
                for h in range(nsplit):
                    xt = xpool.tile([128, kcs, rchunk], mm_dtype,
                                    name="xt", tag="xt")
                    nc.sync.dma_start(
                        out=xt[:],
                        in_=xP[:, rc, h * kcs:(h + 1) * kcs, :])
                    for c in range(kcs):
                        for g in range(nway):
                            nc.tensor.matmul(
                                pt[32 * g:32 * g + R, :],
                                a_tile[:, h * kcs + c, :],
                                xt[:, c, 128 * g:128 * (g + 1)],
                                start=(h == 0 and c == 0),
                                stop=(h == nsplit - 1 and c == kcs - 1),
                                tile_position=(0, 32 * g),
                                skip_group_check=True,
                            )
                tT4 = tpool.tile([128, 128], mm_dtype)
                nc.vector.tensor_copy(tT4[:], pt[:])

                # stage 2: nway concurrent row-strip matmuls
                osbs = [opool.tile([128, D_OUT], out_dtype, name=f"osb{g}",
                                   tag="osb")
                        for g in range(nway)]
                for dc in range(NDC):
                    for g in range(nway):
                        po = popool.tile([128, DC], F32, name=f"po{g}",
                                         tag="po")
                        nc.tensor.matmul(
                            po[:],
                            tT4[32 * g:32 * g + R, :],
                            b4[32 * g:32 * g + R, dc * DC:(dc + 1) * DC],
                            start=True,
                            stop=True,
                            tile_position=(32 * g, 0),
                            skip_group_check=True,
                        )
                        if (dc * nway + g) % 2 == copy_split:
                            nc.scalar.copy(
                                out=osbs[g][:, dc * DC:(dc + 1) * DC],
                                in_=po[:])
                        else:
                            nc.vector.tensor_copy(
                                osbs[g][:, dc * DC:(dc + 1) * DC], po[:])
                for g in range(nway):
                    row0 = rc * rchunk + 128 * g
                    nc.scalar.dma_start(out=out[row0:row0 + 128, :],
                                        in_=osbs[g][:])
    nc.compile()
    return nc


def _get_nc(mm_dtype, out_dtype, rchunk, copy_split, obufs, layout="v2",
            nway=4, nsplit=1, fast_start=0, store_split=1, warmup=0,
            wide_po=False, tail_sync=False, tt_act=False, store_rings=1,
            const_sync=False, dual_load=False):
    key = (str(mm_dtype), str(out_dtype), rchunk, copy_split, obufs, layout,
           nway, nsplit, fast_start, store_split, warmup, wide_po, tail_sync,
           tt_act, store_rings, const_sync, dual_load)
    if key not in _cache:
        if layout == "v3":
            _cache[key] = _build_v3(mm_dtype, out_dtype, nway, copy_split,
                                    nsplit)
        else:
            _cache[key] = _build_v2(mm_dtype, out_dtype, rchunk, copy_split,
                                    obufs, fast_start, store_split, warmup,
                                    wide_po, tail_sync, tt_act, store_rings,
                                    const_sync, dual_load)
    return _cache[key]


def kernel(x, A, B, trace=False, mm_dtype=BF16, out_dtype=BF16, rchunk=256,
           copy_split=3, obufs=4, layout="v2", nway=4, nsplit=1,
           fast_start=2, store_split=2, warmup=0, wide_po=True,
           tail_sync=False, tt_act=False, store_rings=1, const_sync=False,
           dual_load=False):
    x = np.asarray(x, dtype=np.float32)
    A = np.asarray(A, dtype=np.float32)
    B = np.asarray(B, dtype=np.float32)
    npdt = _NPDT[str(mm_dtype)]
    if layout == "v3":
        rchunk = 128 * nway
    nch = RPC // rchunk

    nc = _get_nc(mm_dtype, out_dtype, rchunk, copy_split, obufs, layout,
                 nway, nsplit, fast_start, store_split, warmup, wide_po,
                 tail_sync, tt_act, store_rings, const_sync, dual_load)

    xf = x.reshape(ROWS, D_IN)
    aPh = np.ascontiguousarray(
        A.astype(npdt, copy=False).reshape(KC, 128, R).transpose(1, 0, 2))
    Bh = np.ascontiguousarray(B.astype(npdt, copy=False))

    in_maps = []
    for i in range(N_CORES):
        xs = xf[i * RPC:(i + 1) * RPC]                 # [1024, 4096]
        xPh = np.ascontiguousarray(
            xs.astype(npdt, copy=False)
              .reshape(nch, rchunk, KC, 128)
              .transpose(3, 0, 2, 1))                  # [128, nch, KC, rchunk]
        in_maps.append({"xP": xPh, "aP": aPh, "Bw": Bh})

    res = run_bass_kernel_spmd(nc, in_maps, list(range(N_CORES)), trace=trace)
    outs = [np.asarray(res.results[i]["out"]) for i in range(N_CORES)]
    full = np.concatenate(outs, axis=0).astype(np.float32, copy=False)
    full = full.reshape(BATCH, SEQ, D_OUT)
    if trace:
        kernel.last_exec_time_ns = res.exec_time_ns
        kernel.last_results = res
    return full
